# revision 11
# baseline (speedup 1.0000x reference)
"""Multi-head attention (RoPE, causal) TP-sharded across 8 TRN2 cores.

Sharding: 16 heads -> 2 heads per core (128 of 1024 channels). Each core
computes Q/K/V projections for its channels from the full x, runs causal
attention for its 2 heads x 2 batches, and multiplies by its slice of Wo,
producing a full-shape partial output. Host sums the 8 partials.

Device-side layout: everything transposed ("S^T flash") so softmax reduction
lands on the matmul contraction axis:
  - x fed as xT [1024, 4096] (host-transposed)
  - QT/KT [128ch, 4096] (chan on partitions), RoPE applied in this layout via
    a pair-swap permutation matmul + cos/sin elementwise tables
  - scores^T tiles [128k, 512q] = KT-tile^T @ QT  (contraction d=64/head)
  - E = exp(S^T/8), masked on diagonal blocks
  - numer^T [65, 512q] = V'^T-tile @ E accumulated over k-tiles, where V'
    carries an appended ones-column so row 64 = softmax denominator
  - out[q,m] partial = nattn^T-tile^T @ WoT  (natural orientation)
"""

import json

import numpy as np

import bass_rust
import concourse.bass as bass
import concourse.mybir as mybir
import concourse.tile as tile
from concourse.bass_utils import run_bass_kernel_spmd
from concourse.vector_clock import ScopedClock

f32 = mybir.dt.float32
f32r = mybir.dt.float32r

B, S, DM = 2, 2048, 1024
H, DK = 16, 64
NCORES = 8
CH = DM // NCORES  # 128 channels per core
BS = B * S  # 4096
THETA = 10000.0

USE_F32R = False  # float32r streams 4x faster; flipped after HW accuracy test


# ---------------------------------------------------------------------------
# Workaround: this container's walrus rejects Tile's multi-wait kernel-tail
# Drain ("Too many sync wait commands"). Split it into single-wait drains.
def _split_drain_and_barrier(self, tick_clock, wait_clock):
    gc = tick_clock.global_clock
    vals = json.loads(repr(gc).replace("VectorClock(", "").rstrip(")"))
    for i, v in enumerate(vals):
        if not v:
            continue
        sub = bass_rust.VectorClock([v if j == i else 0 for j in range(len(vals))])
        d = self.nc.sync.drain()
        wait_clock.add_sem_waits(d.ins, ScopedClock({None: sub}))
    self.nc.all_engine_barrier()
    assert self.sems is not None
    popped = self.nc._tile_sem_poison_stack.pop()
    assert popped is self._sem_poison
    self.nc.clear_and_free_semaphores(list(self.sems.allocated().values()))
    self.nc.all_engine_barrier()


tile.TileContext._drain_and_barrier = _split_drain_and_barrier


def _cap_sync_waits(nc, max_waits=1):
    """Same walrus limitation: at most 2 sync waits per instruction. Spill
    excess waits onto preceding same-engine NoOps (engines execute in order,
    so a wait completed on an earlier instruction still gates this one)."""
    n = 0
    for f in nc.m.functions:
        for blk in f.blocks:
            out = []
            for inst in blk.instructions:
                si = inst.sync_info
                waits = list(si.on_wait) if (si is not None and si.on_wait) else []
                lim = max_waits
                if len(waits) > lim:
                    spill, keep = waits[:-lim], waits[-lim:]
                    for ci in range(0, len(spill), lim):
                        chunk = spill[ci : ci + lim]
                        nop = mybir.InstNoOp(
                            name=f"{inst.name}-w{n}", ins=[], outs=[]
                        )
                        nop.engine = inst.engine
                        nop.sync_info = mybir.SyncInfo(on_wait=chunk, on_update=[])
                        out.append(nop)
                        n += 1
                    si.on_wait = keep
                out.append(inst)
            blk.instructions = out
    return n


# ---------------------------------------------------------------------------


def _mm(nc, out, lhsT, rhs, **kw):
    if USE_F32R:
        lhsT = lhsT.bitcast(f32r)
        rhs = rhs.bitcast(f32r)
    return nc.tensor.matmul(out, lhsT, rhs, **kw)


def _build_nc():
    nc = bass.Bass(trn_type="TRN2")

    xT = nc.dram_tensor("xT", [DM, BS], f32, kind="ExternalInput")
    wq = nc.dram_tensor("wq", [DM, CH], f32, kind="ExternalInput")
    wk = nc.dram_tensor("wk", [DM, CH], f32, kind="ExternalInput")
    wv = nc.dram_tensor("wv", [DM, CH], f32, kind="ExternalInput")
    wo = nc.dram_tensor("wo", [CH, DM], f32, kind="ExternalInput")
    cosT = nc.dram_tensor("cosT", [CH, BS], f32, kind="ExternalInput")
    sinT = nc.dram_tensor("sinT", [CH, BS], f32, kind="ExternalInput")
    perm = nc.dram_tensor("perm", [128, 128], f32, kind="ExternalInput")
    ident = nc.dram_tensor("ident", [128, 128], f32, kind="ExternalInput")
    masks = nc.dram_tensor("masks", [128, 2048], f32, kind="ExternalInput")
    out = nc.dram_tensor("out", [BS, DM], f32, kind="ExternalOutput")

    xTv = xT.rearrange("(o p) q -> o p q", p=128)  # [8, 128, 4096]
    wqv = wq.rearrange("(o p) j -> p o j", p=128)  # [128, 8, 128]
    wkv = wk.rearrange("(o p) j -> p o j", p=128)
    wvv = wv.rearrange("(o p) j -> p o j", p=128)
    outv = out.rearrange("(o p) m -> o p m", p=128)  # [32, 128, 1024]

    EXP = mybir.ActivationFunctionType.Exp

    with tile.TileContext(nc) as tc:
        import contextlib

        with contextlib.ExitStack() as ctx:
            consts = ctx.enter_context(tc.tile_pool(name="consts", bufs=1))
            big = ctx.enter_context(tc.tile_pool(name="big", bufs=1))
            xt_p = ctx.enter_context(tc.tile_pool(name="xt", bufs=3))
            e_p = ctx.enter_context(tc.tile_pool(name="e", bufs=4))
            tmp_p = ctx.enter_context(tc.tile_pool(name="tmp", bufs=3))
            rd_p = ctx.enter_context(tc.tile_pool(name="rd", bufs=4))
            rdb_p = ctx.enter_context(tc.tile_pool(name="rdb", bufs=4))
            oe_p = ctx.enter_context(tc.tile_pool(name="oe", bufs=4))
            ps = ctx.enter_context(tc.tile_pool(name="ps", bufs=8, space="PSUM"))

            wo_sb = consts.tile([128, DM], f32)
            nc.sync.dma_start(out=wo_sb, in_=wo[:, :])
            cos_sb = consts.tile([128, BS], f32)
            nc.sync.dma_start(out=cos_sb, in_=cosT[:, :])
            sin_sb = consts.tile([128, BS], f32)
            nc.sync.dma_start(out=sin_sb, in_=sinT[:, :])
            perm_sb = consts.tile([128, 128], f32)
            nc.sync.dma_start(out=perm_sb, in_=perm[:, :])
            id_sb = consts.tile([128, 128], f32)
            nc.sync.dma_start(out=id_sb, in_=ident[:, :])
            mask_sb = consts.tile([128, 2048], f32)
            nc.sync.dma_start(out=mask_sb, in_=masks[:, :])
            w3 = []
            for wi, wview in enumerate((wqv, wkv, wvv)):
                w_sb = consts.tile(
                    [128, 8, 128], f32, name=f"w{wi}_sb", tag=f"w{wi}"
                )
                nc.sync.dma_start(out=w_sb, in_=wview)
                w3.append(w_sb)

            qt = big.tile([128, BS], f32)
            kt = big.tile([128, BS], f32)
            nattn = big.tile([128, BS], f32)
            v_sb = big.tile([128, 2, 32, 65], f32)  # [k-part, head, ktile, d+1]
            nc.vector.memset(v_sb[:, :, :, 64:65], 1.0)
            ones64 = consts.tile([1, 64], f32)
            nc.vector.memset(ones64, 1.0)

            # ---- Q/K/V projections (contraction over d_model on partitions)
            for g in range(4):  # q-groups of 1024 columns
                accs = [
                    ps.tile([128, 512], f32, tag="ps", name=f"acc{t_i}_{qh}")
                    for t_i in range(3)
                    for qh in range(2)
                ]
                for dmt in range(8):
                    xt_t = xt_p.tile([128, 1024], f32, tag="xt", name="xt_t")
                    nc.sync.dma_start(
                        out=xt_t, in_=xTv[dmt, :, g * 1024 : (g + 1) * 1024]
                    )
                    for t_i in range(3):
                        for qh in range(2):
                            _mm(
                                nc,
                                accs[t_i * 2 + qh],
                                lhsT=w3[t_i][:, dmt, :],
                                rhs=xt_t[:, qh * 512 : (qh + 1) * 512],
                                start=(dmt == 0),
                                stop=(dmt == 7),
                            )
                for t_i in range(3):
                    for qh in range(2):
                        acc = accs[t_i * 2 + qh]
                        qg = g * 1024 + qh * 512
                        if t_i == 0:
                            nc.scalar.copy(out=qt[:, qg : qg + 512], in_=acc)
                        elif t_i == 1:
                            nc.scalar.copy(out=kt[:, qg : qg + 512], in_=acc)
                        else:
                            vtmp = tmp_p.tile([128, 512], f32, tag="tmp", name="vtmp")
                            nc.scalar.copy(out=vtmp, in_=acc)
                            for tb in range(4):
                                ktg = qg // 128 + tb
                                tp_ps = ps.tile(
                                    [128, 128], f32, tag="ps", name="tp_ps"
                                )
                                nc.tensor.transpose(
                                    tp_ps,
                                    vtmp[:, tb * 128 : (tb + 1) * 128],
                                    id_sb,
                                )
                                nc.vector.tensor_copy(
                                    out=v_sb[:, 0, ktg, 0:64], in_=tp_ps[:, 0:64]
                                )
                                nc.vector.tensor_copy(
                                    out=v_sb[:, 1, ktg, 0:64], in_=tp_ps[:, 64:128]
                                )

            # ---- RoPE on QT/KT in [chan, seq] layout
            for tn in (qt, kt):
                for blk in range(8):
                    sl = slice(blk * 512, (blk + 1) * 512)
                    pm = ps.tile([128, 512], f32, tag="ps")
                    _mm(nc, pm, lhsT=perm_sb, rhs=tn[:, sl], start=True, stop=True)
                    nc.vector.tensor_mul(tn[:, sl], tn[:, sl], cos_sb[:, sl])
                    tmp = tmp_p.tile([128, 512], f32, tag="tmp")
                    nc.vector.tensor_mul(tmp, pm, sin_sb[:, sl])
                    nc.vector.tensor_add(tn[:, sl], tn[:, sl], tmp)

            # ---- causal attention, 2 heads x 2 batches
            for b in range(2):
                for qb in range(4):
                    qs = b * 2048 + qb * 512
                    nkt = 4 * (qb + 1)
                    numer = [ps.tile([128, 512], f32, tag="ps", name=f"numer{h}") for h in range(2)]
                    for kt_i in range(nkt):
                        ktg = b * 16 + kt_i
                        ks = b * 2048 + kt_i * 128
                        s_ps = [ps.tile([128, 512], f32, tag="ps", name=f"s{h}") for h in range(2)]
                        for h in range(2):
                            _mm(
                                nc,
                                s_ps[h],
                                lhsT=kt[64 * h : 64 * h + 64, ks : ks + 128],
                                rhs=qt[64 * h : 64 * h + 64, qs : qs + 512],
                                start=True,
                                stop=True,
                            )
                        es = []
                        for h in range(2):
                            e = e_p.tile([128, 512], f32, tag="e", name=f"e{h}")
                            nc.scalar.activation(e, s_ps[h], EXP, scale=0.125)
                            if kt_i >= 4 * qb:
                                j = kt_i - 4 * qb
                                nc.vector.tensor_mul(
                                    e, e, mask_sb[:, j * 512 : (j + 1) * 512]
                                )
                            es.append(e)
                        for h in range(2):
                            _mm(
                                nc,
                                numer[h][0:65, :],
                                lhsT=v_sb[:, h, ktg, :],
                                rhs=es[h],
                                start=(kt_i == 0),
                                stop=(kt_i == nkt - 1),
                            )
                    for h in range(2):
                        rd = rd_p.tile([1, 512], f32, tag="rd")
                        nc.vector.reciprocal(rd, numer[h][64:65, :])
                        # broadcast 1/D across 64 partitions via K=1 matmul
                        rdb = ps.tile([128, 512], f32, tag="ps", name="rdb")
                        _mm(nc, rdb[0:64, :], lhsT=ones64, rhs=rd,
                            start=True, stop=True)
                        rdb_sb = rdb_p.tile([64, 512], f32, tag="rdb")
                        nc.scalar.copy(out=rdb_sb, in_=rdb[0:64, :])
                        nc.vector.tensor_mul(
                            nattn[64 * h : 64 * h + 64, qs : qs + 512],
                            numer[h][0:64, :],
                            rdb_sb,
                        )

            # ---- output projection: full-width partial through Wo slice
            for qt_i in range(32):
                for mh in range(2):
                    op = ps.tile([128, 512], f32, tag="ps")
                    _mm(
                        nc,
                        op,
                        lhsT=nattn[:, qt_i * 128 : (qt_i + 1) * 128],
                        rhs=wo_sb[:, mh * 512 : (mh + 1) * 512],
                        start=True,
                        stop=True,
                    )
                    oe = oe_p.tile([128, 512], f32, tag="oe")
                    nc.scalar.copy(out=oe, in_=op)
                    nc.sync.dma_start(
                        out=outv[qt_i, :, mh * 512 : (mh + 1) * 512], in_=oe
                    )

    _cap_sync_waits(nc)
    return nc


def _rope_tables():
    """cos/sin per (seq, freq) matching the fp32 reference computation."""
    half = DK // 2
    try:
        import jax
        import jax.numpy as jnp

        cpu = jax.devices("cpu")[0]
        with jax.default_device(cpu):
            inv_freq = THETA ** (-jnp.arange(half, dtype=jnp.float32) * 2.0 / DK)
            ang = (
                jnp.arange(S, dtype=jnp.int32)[:, None].astype(jnp.float32) * inv_freq
            )
            cos = np.asarray(jax.device_get(jnp.cos(ang)), np.float32)
            sin = np.asarray(jax.device_get(jnp.sin(ang)), np.float32)
    except Exception:
        inv64 = THETA ** (
            -(np.arange(half, dtype=np.float32) * np.float32(2.0) / np.float32(DK))
        ).astype(np.float64)
        ang32 = (
            np.arange(S, dtype=np.float32)[:, None] * inv64.astype(np.float32)[None, :]
        )
        cos = np.cos(ang32.astype(np.float64)).astype(np.float32)
        sin = np.sin(ang32.astype(np.float64)).astype(np.float32)
    return cos, sin  # [S, 32]


def _host_inputs(x, Wq, Wk, Wv, Wo):
    x = np.ascontiguousarray(np.asarray(x, np.float32).reshape(BS, DM))
    xT = np.ascontiguousarray(x.T)  # [1024, 4096]

    cos, sin = _rope_tables()  # [S, 32]
    p = np.arange(CH)
    d = p % DK
    f = d // 2
    sign = np.where(d % 2 == 0, -1.0, 1.0).astype(np.float32)
    cos_p = cos[:, f].T  # [128, S]
    sin_p = sin[:, f].T * sign[:, None]  # [128, S]
    cosT = np.ascontiguousarray(np.tile(cos_p, (1, B)))  # [128, 4096]
    sinT = np.ascontiguousarray(np.tile(sin_p, (1, B)))

    perm = np.zeros((128, 128), np.float32)
    perm[np.arange(128) ^ 1, np.arange(128)] = 1.0
    ident = np.eye(128, dtype=np.float32)

    kl = np.arange(128)[:, None]
    ql = np.arange(512)[None, :]
    masks = np.concatenate(
        [(128 * j + kl <= ql).astype(np.float32) for j in range(4)], axis=1
    )  # [128, 2048]

    shared = dict(xT=xT, cosT=cosT, sinT=sinT, perm=perm, ident=ident, masks=masks)
    in_maps = []
    for c in range(NCORES):
        sl = slice(CH * c, CH * c + CH)
        in_maps.append(
            dict(
                shared,
                wq=np.ascontiguousarray(np.asarray(Wq, np.float32)[sl, :].T),
                wk=np.ascontiguousarray(np.asarray(Wk, np.float32)[sl, :].T),
                wv=np.ascontiguousarray(np.asarray(Wv, np.float32)[sl, :].T),
                wo=np.ascontiguousarray(np.asarray(Wo, np.float32)[:, sl].T),
            )
        )
    return in_maps


_CACHE = {}


def kernel(x, Wq, Wk, Wv, Wo):
    if "nc" not in _CACHE:
        _CACHE["nc"] = _build_nc()
    nc = _CACHE["nc"]
    in_maps = _host_inputs(x, Wq, Wk, Wv, Wo)
    res = run_bass_kernel_spmd(nc, in_maps, core_ids=list(range(NCORES)))
    acc = np.zeros((BS, DM), np.float64)
    for r in res.results:
        acc += r["out"].astype(np.float64)
    return acc.reshape(B, S, DM).astype(np.float32)
